# revision 10
# baseline (speedup 1.0000x reference)
"""Trainium2 Bass kernel for a 3-layer GraphSAGE-GCN (gnn_message_passing).

Math (per layer, commuting the dense matmul through the linear aggregation):
    Y_l   = h_{l-1} @ W_l^T                      (dense per-node matmul)
    h_l   = relu(inv ⊙ (A + I) Y_l)              (edge gather + scatter-add)
with inv = 1/(deg_in + 1); self-loop edges are appended host-side so
(A + I) is a plain edge sum.

Distribution: destination nodes (and their incoming edges) are sharded
across 8 NeuronCores.  Layer-1 input Y1 = x @ W1^T is computed redundantly
in full on every core from a replicated bf16 x^T (cheap: 391 small
matmuls), which removes the first AllGather entirely.  For layers 2/3 the
per-core [6250, D] Y tiles are AllGather'ed in S row-chunks that fire as
the producing layer's dst blocks complete, so the wire time overlaps the
aggregation compute.  y_full for those layers is laid out chunk-major
(chunk, core, row) to keep each collective's output contiguous; the host
remaps gather indices to match.

Scatter-add on device: edges are host-bucketed by (dst block of 128, src
half); per bucket a dma_gather pulls Y[src] rows into SBUF [128 edges x D],
a one-hot selection matrix O[p, j] = (dst_slot[p] == j) is built with one
broadcast is_equal, and PE matmul O^T @ G accumulates the per-dst-block
aggregate in PSUM.  The src half split exists because dma_gather indices
are int16.  The dmod operand is shipped x4-replicated so the is_equal's
broadcast AP keeps a packed (stride-1, n>=2) last dim -- the DVE 2x/4x
fast-path qualifier looks only at the last AP dim of each operand.
"""

import numpy as np

import concourse.bacc as bacc
import concourse.bass as bass
import concourse.mybir as mybir
import concourse.tile as tile
from concourse.bass_utils import run_bass_kernel_spmd
from concourse.masks import make_identity

# Problem constants (hardcoded per harness contract).
N = 50000
DIN = 128
DH = 128
DZ = 64
NCORES = 8
P = 128
NLOC = N // NCORES           # 6250 destination nodes per core
NBLK = (NLOC + P - 1) // P   # 49 dst blocks per core
NGBLK = (N + P - 1) // P     # 391 global blocks (full-Y1 prologue)

F32 = mybir.dt.float32
BF16 = mybir.dt.float16
I16 = mybir.dt.int16

# --- tuning knobs -----------------------------------------------------------
S_CHUNKS = 7                 # AllGather pipeline chunks per layer
ONE_HOT_X4 = True            # x4-replicated dmod for the DVE fast path
NQUEUES = 4
CALL_CHUNKS = 8
GBUFS = 12
PREFETCH_GATHERS = True
PF_DIST = 3
OH_DIST = 1
OPOOL_BUFS = 3
HPOOL_BUFS = 6
AGG_BUFS = 3
XT_TILE = 4                  # global blocks per prologue xT load
MOCK_COLLECTIVES = False
SKIP_GATHER = False
SKIP_MM = False
BUILD_STAGES = 4             # agg0, AGs+agg1, AGs+agg2 grouped below
REPEATS = 1

OHREP = 4                    # dmod replication factor


def _cdiv(a, b):
    return (a + b - 1) // b


def _cdiv_arr(a, b):
    return (np.asarray(a) + b - 1) // b


def _chunk_plan():
    """S row-chunks over the 49 local dst blocks; last block is short."""
    base, rem = NBLK // S_CHUNKS, NBLK % S_CHUNKS
    nblks = [base + (1 if s < rem else 0) for s in range(S_CHUNKS)]
    starts_b = np.concatenate([[0], np.cumsum(nblks)])  # block index bounds
    rows = []
    for s in range(S_CHUNKS):
        b0, b1 = starts_b[s], starts_b[s + 1]
        r0 = b0 * P
        r1 = min(b1 * P, NLOC)
        rows.append((int(r0), int(r1)))
    return nblks, rows


def _remap_chunk_major(ids):
    """Map global node id -> row in the chunk-major y_full layout."""
    _, rows = _chunk_plan()
    starts = np.array([r0 for r0, _ in rows] + [NLOC])
    lens = np.diff(starts)
    cum_all = np.concatenate([[0], np.cumsum(lens * NCORES)])
    c = ids // NLOC
    l = ids - c * NLOC
    s = np.searchsorted(starts, l, side="right") - 1
    o = l - starts[s]
    return cum_all[s] + c * lens[s] + o, cum_all


def _build_group(src_pos, dst, half_at):
    """Bucket edges by (core, src half, dst block); build int16 gather index
    stream, x4-replicated one-hot slot stream, and per-half chunk counts.

    src_pos: position of each edge's source row in the layer's y_full
    layout.  half_at: row splitting lo/hi gather halves (int16 range)."""
    core = dst // NLOC
    ldst = dst - core * NLOC
    blk = ldst // P
    slot = ldst % P
    half = (src_pos >= half_at).astype(np.int64)
    cell = (core * 2 + half) * NBLK + blk
    ncells = NCORES * NBLK * 2

    order = np.argsort(cell, kind="stable")
    cell_s = cell[order]
    half_s = half[order]
    vals = (src_pos[order] - half_s * half_at).astype(np.int16)
    slot_s = slot[order].astype(np.float32)

    counts = np.bincount(cell_s, minlength=ncells).astype(np.int64)
    by_half = counts.reshape(NCORES, 2, NBLK)
    # per-block chunk counts: max over cores (SPMD shares one instruction
    # stream, but per-block variation is compile-time data) -- much less
    # padding than a global max
    c_lo = np.maximum(1, _cdiv_arr(by_half[:, 0, :].max(axis=0), P))
    c_hi = np.maximum(1, _cdiv_arr(by_half[:, 1, :].max(axis=0), P))
    ctot_b = c_lo + c_hi
    start_lo = np.concatenate([[0], np.cumsum(c_lo)])   # chunk ordinal bases
    start_hi = np.concatenate([[0], np.cumsum(c_hi)])
    NCH_LO, NCH_HI = int(start_lo[-1]), int(start_hi[-1])
    dstart = np.concatenate([[0], np.cumsum(ctot_b)])   # dmod col bases
    CPC = int(dstart[-1])
    COLS = (NCH_LO + NCH_HI) * 8

    cell_start = np.zeros(ncells + 1, np.int64)
    cell_start[1:] = np.cumsum(counts)
    q = np.arange(len(cell_s)) - cell_start[cell_s]

    core_s = cell_s // (NBLK * 2)
    b_s = cell_s % NBLK
    chunk0_i = np.where(half_s == 0, start_lo[b_s], NCH_LO + start_hi[b_s])
    chunk0_d = dstart[b_s] + half_s * c_lo[b_s]

    idx_arr = np.zeros((NCORES, 16, COLS), np.int16)
    idx_arr[core_s, q % 16, chunk0_i * 8 + q // 16] = vals
    dmod = np.full((NCORES, P, CPC), -1.0, np.float16)
    dmod[core_s, q % P, chunk0_d + q // P] = slot_s
    dmod4 = np.repeat(dmod, OHREP, axis=2)  # [NC, P, CPC*4]

    return dict(c_lo=tuple(int(v) for v in c_lo),
                c_hi=tuple(int(v) for v in c_hi),
                idx=idx_arr, dmod4=dmod4)


def _preprocess(x, edge_index):
    src = np.asarray(edge_index[0], dtype=np.int64)
    dst = np.asarray(edge_index[1], dtype=np.int64)
    # append self loops; cnt = deg + 1 falls out of the augmented bincount
    loops = np.arange(N, dtype=np.int64)
    src = np.concatenate([src, loops])
    dst = np.concatenate([dst, loops])
    cnt = np.bincount(dst, minlength=N).astype(np.float32)

    ga = _build_group(src, dst, 25000)                   # layer-0 (plain)
    pos12, cum_all = _remap_chunk_major(src)
    half12 = int(cum_all[S_CHUNKS // 2])
    assert half12 < 32768 and (N - half12) < 32768
    gb = _build_group(pos12, dst, half12)                # layers 1-2

    cnt_arr = np.ones((NCORES, NBLK * P), np.float32)
    cnt_arr[:, :NLOC] = cnt.reshape(NCORES, NLOC)
    cnt_arr = np.ascontiguousarray(
        cnt_arr.reshape(NCORES, NBLK, P).transpose(0, 2, 1))  # [NC, P, NBLK]

    x = np.asarray(x, dtype=np.float32)
    x_pad = np.zeros((NGBLK * P, DIN), np.float32)
    x_pad[:N] = x
    xT = np.ascontiguousarray(x_pad.T.astype(np.float16))  # [DIN, NGBLK*P]

    ctm_a = max(la + ha for la, ha in zip(ga["c_lo"], ga["c_hi"]))
    ctm_b = max(lb + hb for lb, hb in zip(gb["c_lo"], gb["c_hi"]))
    CTOTM = max(ctm_a, ctm_b)
    iota = np.tile(np.tile(np.arange(P, dtype=np.float16), (P, 1)),
                   (1, CTOTM))

    return dict(ga=ga, gb=gb, half12=half12, cnt=cnt_arr, xT=xT, iota=iota,
                CTOTM=CTOTM)


def _build(ca_lo, ca_hi, cb_lo, cb_hi, half12):
    # per-block chunk plans (tuples of len NBLK)
    def plan(c_lo, c_hi):
        start_lo = np.concatenate([[0], np.cumsum(c_lo)])
        start_hi = np.concatenate([[0], np.cumsum(c_hi)])
        dstart = np.concatenate(
            [[0], np.cumsum(np.asarray(c_lo) + np.asarray(c_hi))])
        return dict(c_lo=c_lo, c_hi=c_hi,
                    start_lo=start_lo, start_hi=start_hi, dstart=dstart,
                    NCH_LO=int(start_lo[-1]), NCH_HI=int(start_hi[-1]),
                    CPC=int(dstart[-1]))
    pa, pb = plan(ca_lo, ca_hi), plan(cb_lo, cb_hi)
    CTOTM = max(max(l + h for l, h in zip(ca_lo, ca_hi)),
                max(l + h for l, h in zip(cb_lo, cb_hi)))
    COLS_A = (pa["NCH_LO"] + pa["NCH_HI"]) * 8
    COLS_B = (pb["NCH_LO"] + pb["NCH_HI"]) * 8

    nc = bacc.Bacc("TRN2", target_bir_lowering=False, debug=False,
                   num_devices=NCORES, num_swdge_queues=NQUEUES)

    xT_d = nc.dram_tensor("xT", [DIN, NGBLK * P], BF16, kind="ExternalInput")
    idxa_d = nc.dram_tensor("idxa", [16, COLS_A], I16, kind="ExternalInput")
    idxb_d = nc.dram_tensor("idxb", [16, COLS_B], I16, kind="ExternalInput")
    dmoda_d = nc.dram_tensor("dmoda", [P, pa["CPC"] * OHREP], BF16,
                             kind="ExternalInput")
    dmodb_d = nc.dram_tensor("dmodb", [P, pb["CPC"] * OHREP], BF16,
                             kind="ExternalInput")
    cnt_d = nc.dram_tensor("cnt", [P, NBLK], F32, kind="ExternalInput")
    iota_d = nc.dram_tensor("iota", [P, CTOTM * P], BF16,
                            kind="ExternalInput")
    w1t_d = nc.dram_tensor("w1t", [DIN, DH], BF16, kind="ExternalInput")
    w2t_d = nc.dram_tensor("w2t", [DH, DH], F32, kind="ExternalInput")
    w3t_d = nc.dram_tensor("w3t", [DH, DZ], F32, kind="ExternalInput")
    out_d = nc.dram_tensor("out", [NLOC, DZ], F32, kind="ExternalOutput")

    # Y rows are padded to 128 bf16 (256B, the dma_gather minimum elem);
    # layer-3 cols 64:128 are never written or read.
    y1_full = nc.dram_tensor("y1full", [N, DH], BF16)       # per-core local
    y_loc = [nc.dram_tensor(f"y{l}loc", [NLOC, DH], BF16) for l in (2, 3)]
    y_full = [nc.dram_tensor(f"y{l}full", [N, DH], BF16, addr_space="Shared")
              for l in (2, 3)]

    rows_of = [min(P, NLOC - b * P) for b in range(NBLK)]
    grows_of = [min(P, N - b * P) for b in range(NGBLK)]
    nblks_s, rows_s = _chunk_plan()
    starts_b = np.concatenate([[0], np.cumsum(nblks_s)])
    lens_s = [r1 - r0 for r0, r1 in rows_s]
    cum_all = np.concatenate([[0], np.cumsum([l * NCORES for l in lens_s])])

    with tile.TileContext(nc) as tc:
        with (
            tc.tile_pool(name="pers", bufs=1) as pers,
            tc.tile_pool(name="gpool", bufs=GBUFS) as gpool,
            tc.tile_pool(name="opool", bufs=OPOOL_BUFS) as opool,
            tc.tile_pool(name="hpool", bufs=HPOOL_BUFS) as hpool,
            tc.tile_pool(name="agg_ps", bufs=AGG_BUFS, space="PSUM") as agg_pp,
            tc.tile_pool(name="pro_ps", bufs=2, space="PSUM") as pro_pp,
            tc.tile_pool(name="tr_ps", bufs=2, space="PSUM") as tr_pp,
            tc.tile_pool(name="y_ps", bufs=1, space="PSUM") as y_pp,
        ):
            # --- persistent tiles ------------------------------------------
            # The Q7 descriptor generators read the index stream through
            # 16-partition groups (cpu0 rx: 0-15, cpu1 tx: 16-31, ...) --
            # replicate the indices into all 8 groups.
            idxa_sb = pers.tile([P, COLS_A], I16)
            idxb_sb = pers.tile([P, COLS_B], I16)
            for gidx in range(8):
                sl = slice(gidx * 16, (gidx + 1) * 16)
                nc.sync.dma_start(idxa_sb[sl, :], idxa_d[:, :])
                nc.sync.dma_start(idxb_sb[sl, :], idxb_d[:, :])
            dmoda_sb = pers.tile([P, pa["CPC"] * OHREP], BF16)
            nc.sync.dma_start(dmoda_sb[:], dmoda_d[:, :])
            dmodb_sb = pers.tile([P, pb["CPC"] * OHREP], BF16)
            nc.sync.dma_start(dmodb_sb[:], dmodb_d[:, :])
            iota_sb = pers.tile([P, CTOTM * P], BF16)
            nc.sync.dma_start(iota_sb[:], iota_d[:, :])
            cnt_sb = pers.tile([P, NBLK], F32)
            nc.sync.dma_start(cnt_sb[:], cnt_d[:, :])
            inv_sb = pers.tile([P, NBLK], F32)
            nc.vector.reciprocal(inv_sb[:], cnt_sb[:])
            w1t_sb = pers.tile([DIN, DH], BF16)
            nc.sync.dma_start(w1t_sb[:], w1t_d[:, :])
            w2t_sb = pers.tile([DH, DH], F32)
            nc.sync.dma_start(w2t_sb[:], w2t_d[:, :])
            w3t_sb = pers.tile([DH, DZ], F32)
            nc.sync.dma_start(w3t_sb[:], w3t_d[:, :])
            ident = pers.tile([P, P], F32)
            make_identity(nc, ident[:])

            call_no = [0]

            def prologue():
                # full Y1 = x @ W1^T on every core (replicated, no
                # collective).  XT_TILE blocks share one xT load, one PSUM
                # bank, one ACT copy and one batched HWDGE write -- the
                # per-transfer fixed costs dominate at this size.
                for gb0 in range(0, NGBLK, XT_TILE):
                    nb = min(XT_TILE, NGBLK - gb0)
                    xt = hpool.tile([P, XT_TILE * P], BF16, tag="xt")
                    nc.sync.dma_start(xt[:, :nb * P],
                                      xT_d[:, gb0 * P:(gb0 + nb) * P])
                    y_ps = pro_pp.tile([P, XT_TILE * DH], F32, tag="pro")
                    for g in range(gb0, gb0 + nb):
                        j = g - gb0
                        nc.tensor.matmul(y_ps[:, j * DH:(j + 1) * DH],
                                         lhsT=xt[:, j * P:(j + 1) * P],
                                         rhs=w1t_sb[:], start=True, stop=True)
                    y_sb = hpool.tile([P, XT_TILE * DH], BF16, tag="proy")
                    nc.scalar.copy(y_sb[:, :nb * DH], y_ps[:, :nb * DH])
                    if grows_of[gb0 + nb - 1] == P:
                        nc.sync.dma_start(
                            y1_full[gb0 * P:(gb0 + nb) * P, :].rearrange(
                                "(g p) d -> p g d", g=nb),
                            y_sb[:, :nb * DH].rearrange(
                                "p (g d) -> p g d", g=nb))
                    else:
                        for g in range(gb0, gb0 + nb):
                            j = g - gb0
                            r = grows_of[g]
                            nc.sync.dma_start(
                                y1_full[g * P:g * P + r, :],
                                y_sb[:r, j * DH:(j + 1) * DH])

            def allgather_chunk(li, s):
                # li: 0 -> y2, 1 -> y3 (index into y_loc / y_full)
                r0, r1 = rows_s[s]
                dst0 = int(cum_all[s])
                ln = lens_s[s]
                if MOCK_COLLECTIVES:
                    nc.sync.dma_start(y_full[li][dst0:dst0 + ln, :],
                                      y_loc[li][r0:r1, :])
                    return
                nc.gpsimd.collective_compute(
                    "AllGather", mybir.AluOpType.bypass,
                    ins=[y_loc[li][r0:r1, :]],
                    outs=[y_full[li][dst0:dst0 + NCORES * ln, :]],
                    replica_groups=[list(range(NCORES))])

            def agg_layer(li, D, w_next_sb, D_next):
                """li = 0,1,2.  Gathers from y1_full (li=0) or y_full[li-1];
                writes y_loc[li] blocks + fires chunked AllGathers (li<2) or
                writes out (li=2)."""
                EL = DH
                if li == 0:
                    yf = y1_full
                    pl = pa
                    idx_sb, dmod_sb = idxa_sb, dmoda_sb
                    src_ap = [yf[0:25000, :], yf[25000:N, :]]
                else:
                    yf = y_full[li - 1]
                    pl = pb
                    idx_sb, dmod_sb = idxb_sb, dmodb_sb
                    src_ap = [yf[0:half12, :], yf[half12:N, :]]
                c_lo, c_hi = pl["c_lo"], pl["c_hi"]
                start_h = [pl["start_lo"], pl["start_hi"]]
                dstart = pl["dstart"]
                n_ch = [pl["NCH_LO"], pl["NCH_HI"]]
                col0 = [0, pl["NCH_LO"] * 8]
                tiles = [[], []]
                CALL_C = CALL_CHUNKS

                def ensure_call(h, o):
                    k = o // CALL_C
                    while len(tiles[h]) <= k:
                        kk = len(tiles[h])
                        nch = min(CALL_C, n_ch[h] - kk * CALL_C)
                        g = gpool.tile([P, nch * EL], BF16, tag=f"g{h}")
                        c0 = col0[h] + kk * CALL_C * 8
                        if not SKIP_GATHER:
                            nc.gpsimd.dma_gather(
                                g[:].rearrange("p (c d) -> p c d", d=EL),
                                src_ap[h], idx_sb[:, c0:c0 + nch * 8],
                                nch * P, nch * P, EL,
                                queue_num=call_no[0] % NQUEUES)
                        else:
                            nc.vector.memset(g[:, :P], 0)
                        call_no[0] += 1
                        tiles[h].append(g)
                    return tiles[h][k], (o % CALL_C)

                def emit_oh(b):
                    ct = c_lo[b] + c_hi[b]
                    d0 = int(dstart[b])
                    oh = opool.tile([P, CTOTM * P], BF16, tag="oh")
                    dm = dmod_sb[:, d0 * OHREP:(d0 + ct) * OHREP]
                    dm_ap = (dm.rearrange("p (k f) -> p k f", f=OHREP)
                             .unsqueeze(2)
                             .to_broadcast([P, ct, P // OHREP, OHREP]))
                    nc.vector.tensor_tensor(
                        out=oh[:, :ct * P].rearrange(
                            "p (k j f) -> p k j f", f=OHREP, j=P // OHREP),
                        in0=iota_sb[:, :ct * P].rearrange(
                            "p (k j f) -> p k j f", f=OHREP, j=P // OHREP),
                        in1=dm_ap,
                        op=mybir.AluOpType.is_equal)
                    return oh

                def emit_tail(b, agg):
                    r = rows_of[b]
                    h_sb = hpool.tile([P, D], F32, tag="hsb")
                    nc.scalar.activation(
                        h_sb[:], agg[:], mybir.ActivationFunctionType.Relu,
                        scale=inv_sb[:, b:b + 1])
                    if w_next_sb is None:
                        nc.sync.dma_start(out_d[b * P:b * P + r, :],
                                          h_sb[:r, :])
                    else:
                        hT_ps = tr_pp.tile([P, P], F32, tag="htps")
                        nc.tensor.transpose(hT_ps[:], h_sb[:], ident[:])
                        hT_sb = hpool.tile([P, P], F32, tag="htsb")
                        nc.scalar.copy(hT_sb[:], hT_ps[:])
                        y_ps = y_pp.tile([P, D_next], F32, tag="yps")
                        nc.tensor.matmul(y_ps[:], lhsT=hT_sb[:],
                                         rhs=w_next_sb[:], start=True,
                                         stop=True)
                        y_sb = hpool.tile([P, D_next], BF16, tag="ysb")
                        nc.vector.tensor_copy(y_sb[:], y_ps[:])
                        r2 = rows_of[b]
                        nc.sync.dma_start(
                            y_loc[li][b * P:b * P + r2, 0:D_next],
                            y_sb[:r2, :])
                    # fire the AllGather chunk whose last block just wrote
                    if li < 2:
                        for s in range(S_CHUNKS):
                            if starts_b[s + 1] - 1 == b:
                                allgather_chunk(li, s)

                def prefetch_gathers(b2):
                    if b2 >= NBLK:
                        return
                    for h in (0, 1):
                        ensure_call(h, min(int(start_h[h][b2 + 1]) - 1,
                                           n_ch[h] - 1))

                oh_q = [emit_oh(i) for i in range(min(OH_DIST, NBLK))]
                if PREFETCH_GATHERS:
                    prefetch_gathers(0)
                pending = None
                for b in range(NBLK):
                    if b + OH_DIST < NBLK:
                        oh_q.append(emit_oh(b + OH_DIST))
                    if PREFETCH_GATHERS:
                        prefetch_gathers(b + PF_DIST)
                    if pending is not None:
                        emit_tail(*pending)
                    oh_cur = oh_q.pop(0)
                    agg = agg_pp.tile([P, D], F32, tag="agg")
                    ct = c_lo[b] + c_hi[b]
                    for ci in range(ct):
                        h = 0 if ci < c_lo[b] else 1
                        c = ci if ci < c_lo[b] else ci - c_lo[b]
                        o = int(start_h[h][b]) + c
                        g, pos = ensure_call(h, o)
                        if SKIP_MM and ci > 0:
                            continue
                        nc.tensor.matmul(
                            agg[:], lhsT=oh_cur[:, ci * P:(ci + 1) * P],
                            rhs=g[:, pos * EL:pos * EL + D],
                            start=(ci == 0),
                            stop=(ci == ct - 1 or SKIP_MM))
                    pending = (b, agg)
                emit_tail(*pending)

            stages = [
                prologue,
                lambda: agg_layer(0, DH, w2t_sb, DH),
                lambda: agg_layer(1, DH, w3t_sb, DZ),
                lambda: agg_layer(2, DZ, None, None),
            ]
            for st in stages[:BUILD_STAGES]:
                st()
            for _ in range(REPEATS - 1):
                for st in stages[:BUILD_STAGES]:
                    st()

    nc.compile()
    return nc


_cache = {}


def _get_nc(key):
    if key not in _cache:
        _cache[key] = _build(*key)
    return _cache[key]


def nc_key(prep):
    ga, gb = prep["ga"], prep["gb"]
    return (ga["c_lo"], ga["c_hi"], gb["c_lo"], gb["c_hi"], prep["half12"])


def make_in_maps(prep, W1, W2, W3):
    ga, gb = prep["ga"], prep["gb"]
    w1t = np.ascontiguousarray(np.asarray(W1, np.float32).T
                               .astype(np.float16))
    w2t = np.ascontiguousarray(np.asarray(W2, np.float32).T)
    w3t = np.ascontiguousarray(np.asarray(W3, np.float32).T)
    return [{
        "xT": prep["xT"],
        "idxa": ga["idx"][c], "idxb": gb["idx"][c],
        "dmoda": ga["dmod4"][c], "dmodb": gb["dmod4"][c],
        "cnt": prep["cnt"][c],
        "iota": prep["iota"],
        "w1t": w1t, "w2t": w2t, "w3t": w3t,
    } for c in range(NCORES)]


def kernel(x, edge_index, W1, W2, W3, _trace=False):
    prep = _preprocess(x, edge_index)
    nc = _get_nc(nc_key(prep))
    in_maps = make_in_maps(prep, W1, W2, W3)
    res = run_bass_kernel_spmd(nc, in_maps, list(range(NCORES)),
                               trace=_trace)
    out = np.concatenate([res.results[c]["out"] for c in range(NCORES)],
                         axis=0).astype(np.float32)
    if _trace:
        kernel._last_results = res
    return out
